# revision 1
# baseline (speedup 1.0000x reference)
"""Bass/Tile kernel for nn_MixtureOfDepth (moe_routing) on TRN2.

Reference semantics (per batch row b of x [B, S, D]):
  logits = x @ Wg  (+bg, softmax: both monotone -> irrelevant for top-k set)
  sel    = top-CAP tokens of the row by logit
  out    = x, but selected rows replaced by FFN(x_sel) = gelu(x W1 + b1) W2 + b2

Sharding: 8 cores; core c handles batch row b=c//2 and the rank-slice
[h*TSEL, (h+1)*TSEL) (h=c%2) of that row's top-CAP token list (TSEL=CAP/2).
Each core receives the full row (for routing), computes the exact fp32 router
on device (integer bisection for the CAP-th largest logit), compacts the
selected token ids with a matmul-based argfind, gathers the selected rows by
indirect DMA, runs the FFN in bf16, and outputs its y block [TSEL, D] plus the
token ids [TSEL]. The host overlays y rows onto x to build the full output.

All device ops are standard BIR ops (PE matmul/transpose, DVE, ACT, DMA) --
no gpsimd ucode extended instructions (unavailable on this image).
"""

import sys

sys.path.insert(0, "/opt/trn_rl_repo")

from contextlib import ExitStack

import numpy as np

import concourse.bass as bass
import concourse.mybir as mybir
import concourse.tile as tile
from concourse import bacc

F32 = mybir.dt.float32
I32 = mybir.dt.int32
BF16 = mybir.dt.bfloat16
AF = mybir.ActivationFunctionType
OP = mybir.AluOpType

P = 128


def build_bass(S, D, DFF, CAP, TSEL, CHUNK, debug_stop=0):
    """Build the Bass program (one SPMD program; per-core data differs)."""
    assert S % P == 0 and D % P == 0 and DFF % P == 0
    FA = S // P          # A-layout cols (token t = p + P*f)
    FB = S // P          # B-layout cols (token t = FB*p + f)
    KD = D // P          # feature 128-blocks
    MD = DFF // P        # dff 128-blocks
    G = TSEL // P        # gather groups
    NCHUNK = TSEL // CHUNK
    GPC = CHUNK // P     # gather groups per chunk
    N1 = min(512, CHUNK)     # matmul1 moving width
    NN1 = CHUNK // N1
    SUBTOK = min(256, CHUNK)  # matmul2 token sub-chunk
    NSUB = CHUNK // SUBTOK
    MT = SUBTOK // P     # token 128-groups per sub
    ND = (D + 511) // 512  # D col groups for matmul2
    NDW = min(512, D)
    JW = min(1024, TSEL)   # M-matrix compare width per DVE op
    NJW = TSEL // JW

    nc = bacc.Bacc("TRN2", target_bir_lowering=False, debug=False)

    # ---- I/O ----
    x_t = nc.dram_tensor("x", [S, D], F32, kind="ExternalInput")
    w1t_t = nc.dram_tensor("w1t", [MD * KD * P, P], F32, kind="ExternalInput")
    w2_t = nc.dram_tensor("w2", [DFF, D], F32, kind="ExternalInput")
    wgb_t = nc.dram_tensor("wgb", [P, D], F32, kind="ExternalInput")
    b1pm_t = nc.dram_tensor("b1pm", [P, MD], F32, kind="ExternalInput")
    b2b_t = nc.dram_tensor("b2b", [P, D], F32, kind="ExternalInput")
    iotat_t = nc.dram_tensor("iotat", [P, FA], F32, kind="ExternalInput")
    iotaj_t = nc.dram_tensor("iotaj", [P, TSEL], F32, kind="ExternalInput")
    ut_t = nc.dram_tensor("ut", [P, P], F32, kind="ExternalInput")
    identb_t = nc.dram_tensor("identb", [P, P], BF16, kind="ExternalInput")

    y_t = nc.dram_tensor("y", [TSEL, D], F32, kind="ExternalOutput")
    ids_t = nc.dram_tensor("ids", [TSEL], I32, kind="ExternalOutput")

    # internal DRAM scratch for layout bounces
    msk_s = nc.dram_tensor("msk_s", [S], F32)
    pos_s = nc.dram_tensor("pos_s", [S], F32)

    x = x_t.ap()
    with tile.TileContext(nc) as tc, ExitStack() as ctx:
        # ---------- long-lived pools ----------
        cpool = ctx.enter_context(tc.tile_pool(name="consts", bufs=1))
        stream = ctx.enter_context(tc.tile_pool(name="stream", bufs=6))
        idp = ctx.enter_context(tc.tile_pool(name="idp", bufs=1))

        b2b = cpool.tile([P, D], F32)
        nc.sync.dma_start(b2b[:], b2b_t.ap())
        b1pm = cpool.tile([P, MD], F32)
        nc.sync.dma_start(b1pm[:], b1pm_t.ap())
        identb = cpool.tile([P, P], BF16)
        nc.sync.dma_start(identb[:], identb_t.ap())

        ids32 = idp.tile([P, G], I32)

        # =========================================================
        # ROUTING (scoped pools, released before the FFN peak)
        # =========================================================
        with tc.tile_pool(name="rout", bufs=1) as rp, \
             tc.tile_pool(name="junk", bufs=2) as jp, \
             tc.tile_pool(name="srch", bufs=2) as sp, \
             tc.tile_pool(name="mpool", bufs=2) as mp, \
             tc.tile_pool(name="psR", bufs=1, space="PSUM") as psR, \
             tc.tile_pool(name="psS", bufs=1, space="PSUM") as psS:

            wgb = rp.tile([P, D], F32)
            nc.sync.dma_start(wgb[:], wgb_t.ap())
            iotat = rp.tile([P, FA], F32)
            nc.sync.dma_start(iotat[:], iotat_t.ap())
            iotaj = rp.tile([P, TSEL], F32)
            nc.sync.dma_start(iotaj[:], iotaj_t.ap())
            ut = rp.tile([P, P], F32)
            nc.sync.dma_start(ut[:], ut_t.ap())

            # ---- Phase 1: logits (A-layout [P, FA], t = p + P*f) ----
            logA = rp.tile([P, FA], F32)
            for f in range(FA):
                xt = stream.tile([P, D], F32, tag="stream")
                nc.sync.dma_start(xt[:], x[f * P:(f + 1) * P, :])
                junk = jp.tile([P, D], F32, tag="junk")
                nc.vector.scalar_tensor_tensor(
                    out=junk[:], in0=xt[:], scalar=1.0, in1=wgb[:],
                    op0=OP.mult, op1=OP.mult, accum_out=logA[:, f:f + 1])

            if debug_stop == 1:
                nc.sync.dma_start(y_t.ap()[0:P, 0:FA], logA[:])
            if debug_stop == 0 or debug_stop >= 2:
                # ---- Phase 2: int32 sortable keys ----
                bits = logA[:].bitcast(I32)
                neg = rp.tile([P, FA], I32)
                nc.vector.tensor_scalar(out=neg[:], in0=logA[:], scalar1=0.0,
                                        scalar2=None, op0=OP.is_lt)
                flip = rp.tile([P, FA], I32)
                nc.vector.tensor_scalar(out=flip[:], in0=bits, scalar1=0x7FFFFFFF,
                                        scalar2=None, op0=OP.bitwise_xor)
                keys = rp.tile([P, FA], I32)
                nc.vector.tensor_copy(keys[:], bits)
                nc.vector.copy_predicated(keys[:], neg[:], flip[:])

                # ---- Phase 3: bisection for threshold (CAP-th largest) ----
                # Width-based: lo keeps invariant count(keys > lo) >= CAP;
                # candidate mid = lo + 2^(31-i); accept if count stays >= CAP.
                lo = rp.tile([P, 1], I32)
                nc.vector.memset(lo[:], -(2 ** 31))
                onesc = rp.tile([P, 1], F32)
                nc.vector.memset(onesc[:], 1.0)
                ones2d = rp.tile([P, P], F32)
                nc.vector.memset(ones2d[:], 1.0)
                for i in range(32):
                    mid = sp.tile([P, 1], I32, tag="mid")
                    if i == 0:
                        nc.vector.memset(mid[:], 0)
                    else:
                        nc.vector.tensor_scalar(out=mid[:], in0=lo[:],
                                                scalar1=1 << (31 - i),
                                                scalar2=None, op0=OP.add)
                    junk2 = sp.tile([P, FA], F32, tag="junk2")
                    cntp = sp.tile([P, 1], F32, tag="cntp")
                    nc.vector.scalar_tensor_tensor(
                        out=junk2[:], in0=keys[:], scalar=mid[:, 0:1],
                        in1=onesc[:, 0:1].to_broadcast([P, FA]),
                        op0=OP.is_gt, op1=OP.mult, accum_out=cntp[:])
                    tot_ps = psS.tile([P, 1], F32, tag="tot", space="PSUM")
                    nc.tensor.matmul(tot_ps[:], lhsT=ones2d[:], rhs=cntp[:],
                                     start=True, stop=True)
                    cond_i = sp.tile([P, 1], I32, tag="cond_i")
                    nc.vector.tensor_scalar(out=cond_i[:], in0=tot_ps[:],
                                            scalar1=float(CAP), scalar2=None,
                                            op0=OP.is_ge)
                    nc.vector.copy_predicated(lo[:], cond_i[:], mid[:])
                if debug_stop == 2:
                    lo_f = rp.tile([P, 1], F32)
                    nc.vector.tensor_copy(lo_f[:], lo[:])
                    nc.sync.dma_start(y_t.ap()[0:P, 1:2], lo_f[:])
            if debug_stop == 0 or debug_stop >= 3:
                maskA = rp.tile([P, FA], F32)
                nc.vector.tensor_tensor(out=maskA[:], in0=keys[:],
                                        in1=lo[:].to_broadcast([P, FA]), op=OP.is_gt)

                # ---- Phase 4: exclusive rank pos[t] over the row (B-layout) ----
                nc.sync.dma_start(msk_s.ap().rearrange("(f p) -> p f", p=P), maskA[:])
                maskB = rp.tile([P, FB], F32)
                nc.sync.dma_start(maskB[:], msk_s.ap().rearrange("(p f) -> p f", p=P))
                rowtot = rp.tile([P, 1], F32)
                nc.vector.tensor_reduce(out=rowtot[:], in_=maskB[:],
                                        axis=mybir.AxisListType.X, op=OP.add)
                rowcum_ps = psR.tile([P, 1], F32, tag="rowcum", space="PSUM")
                nc.tensor.matmul(rowcum_ps[:], lhsT=ut[:], rhs=rowtot[:],
                                 start=True, stop=True)
                rowcum = rp.tile([P, 1], F32)
                nc.vector.tensor_copy(rowcum[:], rowcum_ps[:])
                # inclusive prefix along free dim (log-doubling, ping-pong)
                c0 = rp.tile([P, FB], F32)
                nc.vector.tensor_copy(c0[:], maskB[:])
                c1 = rp.tile([P, FB], F32)
                cur, nxt = c0, c1
                s = 1
                while s < FB:
                    nc.vector.tensor_copy(nxt[:, 0:s], cur[:, 0:s])
                    nc.vector.tensor_tensor(out=nxt[:, s:FB], in0=cur[:, s:FB],
                                            in1=cur[:, 0:FB - s], op=OP.add)
                    cur, nxt = nxt, cur
                    s *= 2
                posB = rp.tile([P, FB], F32)
                # pos = (incl + rowcum) - mask  (exclusive global rank)
                nc.vector.scalar_tensor_tensor(out=posB[:], in0=cur[:],
                                               scalar=rowcum[:, 0:1], in1=maskB[:],
                                               op0=OP.add, op1=OP.subtract)
                nc.sync.dma_start(pos_s.ap().rearrange("(p f) -> p f", p=P), posB[:])
                posA = rp.tile([P, FA], F32)
                nc.sync.dma_start(posA[:], pos_s.ap().rearrange("(f p) -> p f", p=P))
                if debug_stop == 3:
                    nc.sync.dma_start(y_t.ap()[P:2 * P, 0:FA], posA[:])
            if debug_stop == 0 or debug_stop >= 4:
                # masked pos: pos if selected else -1
                posm = rp.tile([P, FA], F32)
                nc.vector.scalar_tensor_tensor(out=posm[:], in0=posA[:], scalar=1.0,
                                               in1=maskA[:], op0=OP.add, op1=OP.mult)
                nc.vector.tensor_scalar(out=posm[:], in0=posm[:], scalar1=1.0,
                                        scalar2=None, op0=OP.subtract)

                # ---- Phase 5: M-matrix argfind: ids[j] = t, pos[t] == j ----
                # jgrp-major so early id columns unlock gathers before the
                # whole M sweep finishes. Each 128-wide j block accumulates
                # sum_t t * (pos[t]==j) over all kt into its own psum bank.
                JW2 = min(512, TSEL)
                Q = JW2 // P
                with tc.tile_pool(name="psM", bufs=1, space="PSUM") as psM:
                    for j4 in range(TSEL // JW2):
                        col_ps = [psM.tile([P, 1], F32, tag=f"idcol{q}",
                                           name=f"idcol{q}", space="PSUM")
                                  for q in range(Q)]
                        for kt in range(FA):
                            mtile = mp.tile([P, JW2], F32, tag="mtile")
                            nc.vector.tensor_tensor(
                                out=mtile[:],
                                in0=posm[:, kt:kt + 1].to_broadcast([P, JW2]),
                                in1=iotaj[:, j4 * JW2:(j4 + 1) * JW2],
                                op=OP.is_equal)
                            for q in range(Q):
                                nc.tensor.matmul(
                                    col_ps[q][:],
                                    lhsT=mtile[:, q * P:(q + 1) * P],
                                    rhs=iotat[:, kt:kt + 1],
                                    start=(kt == 0), stop=(kt == FA - 1))
                        for q in range(Q):
                            nc.vector.tensor_copy(
                                ids32[:, j4 * Q + q:j4 * Q + q + 1],
                                col_ps[q][:])
                nc.sync.dma_start(ids_t.ap().rearrange("(g p) -> p g", p=P), ids32[:])

        # =========================================================
        # GATHER + TRANSPOSE + FFN (per chunk)
        # (pools created after routing pools close so PSUM/SBUF is reused)
        # =========================================================
        if 1 <= debug_stop <= 4:
            NCHUNK_RUN = 0
        else:
            NCHUNK_RUN = NCHUNK
        selxbp = ctx.enter_context(tc.tile_pool(name="selxb", bufs=2))
        seltp = ctx.enter_context(tc.tile_pool(name="selt", bufs=1))
        htp = ctx.enter_context(tc.tile_pool(name="ht", bufs=1))
        w1bp = ctx.enter_context(tc.tile_pool(name="w1b", bufs=2))
        w2bp = ctx.enter_context(tc.tile_pool(name="w2b", bufs=1))
        yp = ctx.enter_context(tc.tile_pool(name="yp", bufs=2))
        psA = ctx.enter_context(tc.tile_pool(name="psA", bufs=2, space="PSUM"))
        psY = ctx.enter_context(tc.tile_pool(name="psY", bufs=1, space="PSUM"))
        for c in range(NCHUNK_RUN):
            selt = seltp.tile([P, KD, CHUNK], BF16, tag="selt")
            for gc in range(GPC):
                g = c * GPC + gc
                selx = stream.tile([P, D], F32, tag="stream")
                nc.gpsimd.indirect_dma_start(
                    out=selx[:], out_offset=None, in_=x,
                    in_offset=bass.IndirectOffsetOnAxis(ap=ids32[:, g:g + 1],
                                                        axis=0))
                selxb = selxbp.tile([P, D], BF16, tag="selxb")
                nc.scalar.activation(selxb[:], selx[:], AF.Copy)
                for kf in range(KD):
                    tp = psA.tile([P, P], BF16, tag="tp", space="PSUM")
                    nc.tensor.transpose(tp[:], selxb[:, kf * P:(kf + 1) * P],
                                        identb[:])
                    nc.vector.tensor_copy(selt[:, kf, gc * P:(gc + 1) * P], tp[:])

            if debug_stop == 5:
                w = min(CHUNK, D)
                sel_f = yp.tile([P, D], F32, tag="ysb")
                nc.vector.tensor_copy(sel_f[:, 0:w], selt[:, 0, 0:w])
                nc.sync.dma_start(y_t.ap()[c * P:(c + 1) * P, 0:w],
                                  sel_f[:, 0:w])
                continue
            # ---- W2 for this chunk (bf16, two half-tiles) ----
            KCH = MD // 2
            w2h = []
            for half in range(2):
                w2x = w2bp.tile([P, KCH, D], BF16, tag=f"w2h{half}")
                for k in range(KCH):
                    kc = half * KCH + k
                    w2l = stream.tile([P, D], F32, tag="stream")
                    nc.sync.dma_start(w2l[:], w2_t.ap()[kc * P:(kc + 1) * P, :])
                    nc.scalar.activation(w2x[:, k, :], w2l[:], AF.Copy)
                w2h.append(w2x)

            # ---- matmul1 + gelu -> hT [P, MD, CHUNK] bf16 ----
            ht = htp.tile([P, MD, CHUNK], BF16, tag="ht")
            for m in range(MD):
                w1l = stream.tile([P, KD, P], F32, tag="stream")
                nc.sync.dma_start(
                    w1l[:],
                    w1t_t.ap()[m * KD * P:(m + 1) * KD * P, :]
                    .rearrange("(kf kp) mp -> kp kf mp", kf=KD))
                w1b = w1bp.tile([P, KD, P], BF16, tag="w1b")
                nc.scalar.activation(w1b[:], w1l[:], AF.Copy)
                for n in range(NN1):
                    psh = psA.tile([P, N1], F32, tag="psh", space="PSUM")
                    for kf in range(KD):
                        nc.tensor.matmul(
                            psh[:], lhsT=w1b[:, kf, :],
                            rhs=selt[:, kf, n * N1:(n + 1) * N1],
                            start=(kf == 0), stop=(kf == KD - 1))
                    nc.scalar.activation(ht[:, m, n * N1:(n + 1) * N1], psh[:],
                                         AF.Gelu, bias=b1pm[:, m:m + 1])

            # ---- matmul2 + b2 -> y rows ----
            for sub in range(NSUB):
                psy = [[psY.tile([P, NDW], F32, tag=f"psy{mt}{nd}",
                                 name=f"psy{mt}{nd}", space="PSUM")
                        for nd in range(ND)] for mt in range(MT)]
                for kc in range(MD):
                    w2x = w2h[kc // KCH]
                    for mt in range(MT):
                        tc0 = sub * SUBTOK + mt * P
                        for nd in range(ND):
                            nc.tensor.matmul(
                                psy[mt][nd][:],
                                lhsT=ht[:, kc, tc0:tc0 + P],
                                rhs=w2x[:, kc % KCH, nd * NDW:(nd + 1) * NDW],
                                start=(kc == 0), stop=(kc == MD - 1))
                for mt in range(MT):
                    ysb = yp.tile([P, D], F32, tag="ysb")
                    for nd in range(ND):
                        nc.vector.scalar_tensor_tensor(
                            out=ysb[:, nd * NDW:(nd + 1) * NDW],
                            in0=psy[mt][nd][:], scalar=1.0,
                            in1=b2b[:, nd * NDW:(nd + 1) * NDW],
                            op0=OP.mult, op1=OP.add)
                    r0 = c * CHUNK + sub * SUBTOK + mt * P
                    nc.sync.dma_start(y_t.ap()[r0:r0 + P, :], ysb[:])

    nc.compile()
    return nc


def make_consts(S, D, DFF, TSEL, W1, b1, b2, Wg):
    """Host-side constant tensors shared by every core (except iotaj)."""
    import ml_dtypes
    FA = S // P
    KD = D // P
    MD = DFF // P
    w1t = np.ascontiguousarray(
        W1.reshape(KD, P, MD, P).transpose(2, 0, 1, 3)
    ).reshape(MD * KD * P, P).astype(np.float32)
    return {
        "w1t": w1t,
        "wgb": np.broadcast_to(Wg.reshape(1, D), (P, D)).astype(np.float32).copy(),
        "b1pm": np.ascontiguousarray(b1.reshape(MD, P).T).astype(np.float32),
        "b2b": np.broadcast_to(b2.reshape(1, D), (P, D)).astype(np.float32).copy(),
        "iotat": (np.arange(P)[:, None] + P * np.arange(FA)[None, :]).astype(np.float32),
        "ut": np.triu(np.ones((P, P), np.float32), k=1),
        "identb": np.eye(P, dtype=ml_dtypes.bfloat16),
    }


def run_cores(nc, in_maps, core_ids, trace=False):
    from concourse.bass_utils import run_bass_kernel_spmd
    return run_bass_kernel_spmd(nc, in_maps, core_ids, trace=trace)


# ---------------- full-size entry point ----------------
B_, S_, D_ = 4, 8192, 1024
CAP_ = 4096
DFF_ = 4 * D_
TSEL_ = CAP_ // 2
CHUNK_ = 1024

_cached = {}


def kernel(x, Wg, bg, W1, b1, W2, b2):
    import ml_dtypes  # noqa

    x = np.asarray(x, dtype=np.float32)
    Wg = np.asarray(Wg, dtype=np.float32)
    W1 = np.asarray(W1, dtype=np.float32)
    b1 = np.asarray(b1, dtype=np.float32)
    W2 = np.asarray(W2, dtype=np.float32)
    b2 = np.asarray(b2, dtype=np.float32)

    if "nc" not in _cached:
        _cached["nc"] = build_bass(S_, D_, DFF_, CAP_, TSEL_, CHUNK_)
    nc = _cached["nc"]

    consts = make_consts(S_, D_, DFF_, TSEL_, W1, b1, b2, Wg)
    consts["w2"] = W2.astype(np.float32)

    in_maps = []
    for c in range(8):
        b, h = c // 2, c % 2
        m = dict(consts)
        m["x"] = np.ascontiguousarray(x[b])
        m["iotaj"] = np.broadcast_to(
            (np.arange(TSEL_) + h * TSEL_).astype(np.float32)[None, :],
            (P, TSEL_)).copy()
        in_maps.append(m)

    res = run_cores(nc, in_maps, list(range(8)))
    out = x.copy()
    for c in range(8):
        b = c // 2
        ids = res.results[c]["ids"]
        y = res.results[c]["y"]
        out[b, ids.astype(np.int64)] = y
    return out

